# revision 38
# baseline (speedup 1.0000x reference)
"""Trainium2 Bass kernel for the octonion causal self-attention block.

Strategy (8 NeuronCores, SPMD):
  Each core owns one octonion component c (= heads 2c, 2c+1).
  - Host prep: ternary-quantize weights exactly as the reference does.
    q/k projections run in fp8 DoubleRow mode (2x PE throughput): the
    ternary weights are {-1,0,+1} -- exact in fp8 -- and x is cast to
    fp8 e4m3; the scalar scales (s_q*s_k/sqrt(HD)) fold into the rope
    cos/sin tables of q. v and o projections stay bf16 (fp8 there
    breaks the 2e-2 accuracy budget; verified by simulation).
  - Device phase 1: projections qT/kT (channel-major, fp8 DoubleRow) +
    v (natural, bf16), streaming xt once; RoPE applied on the fly. The
    v projection lags one chunk behind q/k so its (bigger) bf16 xt
    stream has time to arrive during the DMA-bound ramp.
  - Device phase 2: causal attention per (batch, head): S^T = K^T Q,
    exp (scores O(1) bounded), triangular mask on the diagonal tile,
    P^T V with an appended ones-column giving the softmax denominator,
    normalize, PE-transpose back to channel-major yT (kept in SBUF).
    scores(j) and PV(j-1) interleave so the PE keeps working while the
    ACT engine (the exp stream, ~1.7x slower than the scores matmuls)
    catches up.
  - Device phase 3: partial output projection with only local rows of
    Wo; the host sums the 8 partial [C, B*T] outputs while unsharding.
"""

import numpy as np
import ml_dtypes

import concourse.bass as bass
import concourse.tile as tile
from concourse import bacc, mybir
from concourse.bass_utils import run_bass_kernel_spmd
from concourse.masks import make_identity

# ---------------------------------------------------------------- problem dims
B, T_FULL, C, H = 2, 2048, 2048, 16
HD = C // H          # 128
P = C // 8           # 256
N_CORES = 8
KT = C // 128        # 16 contraction k-tiles

OCT_SIGN = np.array([
    [1, 1, 1, 1, 1, 1, 1, 1],
    [1,-1, 1,-1, 1,-1,-1, 1],
    [1,-1,-1, 1, 1, 1,-1,-1],
    [1, 1,-1,-1, 1,-1, 1,-1],
    [1,-1,-1,-1,-1, 1, 1, 1],
    [1, 1,-1, 1,-1,-1,-1, 1],
    [1, 1, 1,-1,-1, 1,-1,-1],
    [1,-1, 1, 1,-1,-1, 1,-1]], dtype=np.float32)
OCT_IDX = np.array([
    [0,1,2,3,4,5,6,7],
    [1,0,3,2,5,4,7,6],
    [2,3,0,1,6,7,4,5],
    [3,2,1,0,7,6,5,4],
    [4,5,6,7,0,1,2,3],
    [5,4,7,6,1,0,3,2],
    [6,7,4,5,2,3,0,1],
    [7,6,5,4,3,2,1,0]], dtype=np.int32)
_COMB = np.zeros((8, 8, 8), np.float32)
for _i in range(8):
    for _j in range(8):
        _COMB[OCT_IDX[_i, _j], _i, _j] = OCT_SIGN[_i, _j]

BF16 = ml_dtypes.bfloat16
FP8 = ml_dtypes.float8_e4m3   # TRN FP8_EXP4 (max +-240)


# ------------------------------------------------------------------- host prep
def _ternary_unit_scale(W: np.ndarray) -> tuple[np.ndarray, float]:
    """Reference's ternary quantization, split into unit {-1,0,1} + scale."""
    try:
        import jax
        import jax.numpy as jnp
        cpu = jax.local_devices(backend="cpu")[0]
        with jax.default_device(cpu):
            Wj = jnp.asarray(W)
            s = jnp.mean(jnp.abs(Wj)) + 1e-8
            u = jnp.round(jnp.clip(Wj / s, -1.0, 1.0))
            return np.asarray(u, np.float32), float(s)
    except Exception:
        s = np.float32(np.mean(np.abs(W.astype(np.float32)))) + np.float32(1e-8)
        return np.rint(np.clip(W / s, -1.0, 1.0)).astype(np.float32), float(s)


def _build_w_eff_unit(W: np.ndarray) -> tuple[np.ndarray, float]:
    """[8,P,P] weights -> effective unit-ternary [C, C] + scalar scale."""
    u, s = _ternary_unit_scale(W)
    eff = np.einsum("kij,ipq->jpkq", _COMB, u).reshape(C, C).astype(np.float32)
    return eff, s


def _rope_colperm() -> np.ndarray:
    """colperm[new] = old: within each head, [re0..re63 | im0..im63]."""
    perm = np.zeros(C, dtype=np.int64)
    for h in range(H):
        base = h * HD
        for r in range(HD // 2):
            perm[base + r] = base + 2 * r
            perm[base + HD // 2 + r] = base + 2 * r + 1
    return perm


def _dup_half_tables(cos: np.ndarray, sin: np.ndarray, scale: float):
    """RoPE tables in duplicated-half layout [128, T], bf16."""
    Tn = cos.shape[0]
    cosd = np.empty((128, Tn), np.float32)
    cosd[0:64] = cos.T
    cosd[64:128] = cos.T
    sind = np.empty((128, Tn), np.float32)
    sind[0:64] = -sin.T
    sind[64:128] = sin.T
    return (cosd * scale).astype(BF16), (sind * scale).astype(BF16)


def prep_inputs(inputs: dict, T: int) -> list[dict]:
    """Build the 8 per-core input maps from the full problem inputs."""
    NT = B * T
    x = np.asarray(inputs["x"], np.float32)[:, :T, :]
    cos = np.asarray(inputs["freqs_cos"], np.float32)[:T]   # [T, 64]
    sin = np.asarray(inputs["freqs_sin"], np.float32)[:T]

    wq_eff, s_q = _build_w_eff_unit(np.asarray(inputs["wq"], np.float32))
    wk_eff, s_k = _build_w_eff_unit(np.asarray(inputs["wk"], np.float32))
    wv_eff, s_v = _build_w_eff_unit(np.asarray(inputs["wv"], np.float32))
    wo_eff, s_o = _build_w_eff_unit(np.asarray(inputs["wo"], np.float32))
    # all four weight sets stay unit-ternary {-1,0,+1} (exact in fp8).
    # s_q*s_k/sqrt(HD) folds into the q PSUM->SBUF copy scale (compile-time
    # constant); s_v*s_o folds into the host-side unshard sum.

    perm = _rope_colperm()
    wq_eff = wq_eff[:, perm]
    wk_eff = wk_eff[:, perm]
    beta = float(s_q) * float(s_k) / float(np.sqrt(HD))
    out_scale = float(s_v) * float(s_o)

    xf = x.reshape(NT, C).T          # [C, NT]
    xt = np.ascontiguousarray(xf.reshape(KT, 128, NT).astype(BF16))
    xt8 = np.ascontiguousarray(xf.reshape(KT, 128, NT).astype(FP8))

    cosd, sind = _dup_half_tables(cos, sin, 1.0)

    tri = np.triu(np.ones((128, 128), np.float32)).astype(BF16)  # [s,q] s<=q

    def blocks8(w_eff: np.ndarray, c: int) -> np.ndarray:
        blk = w_eff[:, c * P:(c + 1) * P]                  # [C, 256]
        return np.ascontiguousarray(blk.reshape(KT, 128, P).astype(FP8))

    in_maps = []
    for c in range(N_CORES):
        wo_rows = np.ascontiguousarray(
            wo_eff[c * P:(c + 1) * P, :].reshape(2, 128, C).astype(FP8))
        in_maps.append({
            "xt": xt,
            "xt8": xt8,
            "wq": blocks8(wq_eff, c),
            "wk": blocks8(wk_eff, c),
            "wv": blocks8(wv_eff, c),
            "wo": wo_rows,
            "cosd": cosd, "sind": sind,
            "tri": tri,
        })
    return in_maps, beta, out_scale


# ------------------------------------------------------------- device program
def build_nc(T: int = T_FULL, n_cores: int = N_CORES, beta: float = 1.0):
    NT = B * T
    ST = T // 128            # s-tiles per batch
    NST = NT // 128
    TCH = min(512, T)        # token chunk; must not cross a batch boundary
    NCH = NT // TCH
    bf16 = mybir.dt.bfloat16
    fp8 = mybir.dt.float8e4
    f32 = mybir.dt.float32
    DR = mybir.MatmulPerfMode.DoubleRow

    nc = bacc.Bacc("TRN2", target_bir_lowering=False, debug=False,
                   num_devices=n_cores)

    xt_d = nc.dram_tensor("xt", [KT, 128, NT], bf16, kind="ExternalInput")
    xt8_d = nc.dram_tensor("xt8", [KT, 128, NT], fp8, kind="ExternalInput")
    wq_d = nc.dram_tensor("wq", [KT, 128, P], fp8, kind="ExternalInput")
    wk_d = nc.dram_tensor("wk", [KT, 128, P], fp8, kind="ExternalInput")
    wv_d = nc.dram_tensor("wv", [KT, 128, P], fp8, kind="ExternalInput")
    wo_d = nc.dram_tensor("wo", [2, 128, C], fp8, kind="ExternalInput")
    cos_d = nc.dram_tensor("cosd", [128, T], bf16, kind="ExternalInput")
    sin_d = nc.dram_tensor("sind", [128, T], bf16, kind="ExternalInput")
    tri_d = nc.dram_tensor("tri", [128, 128], bf16, kind="ExternalInput")
    out_d = nc.dram_tensor("outt", [C, NT], bf16, kind="ExternalOutput")

    with tile.TileContext(nc) as tc:
        with (
            tc.tile_pool(name="consts", bufs=1) as consts,
            tc.tile_pool(name="persist", bufs=1) as persist,
        ):
            # ================= phase 1: projections + rope =================
            with (
                nc.named_scope("proj"),
                tc.tile_pool(name="xts", bufs=3) as xts_pool,
                tc.tile_pool(name="xt8s", bufs=2) as xt8s_pool,
                tc.tile_pool(name="rope", bufs=6) as rope_pool,
                tc.tile_pool(name="ps1", bufs=3, space="PSUM") as ps1,
                tc.tile_pool(name="psv", bufs=3, space="PSUM") as psv,
            ):
                # ---- resident constants
                wq_s = consts.tile([128, KT, P], fp8, tag="wq")
                wk_s = consts.tile([128, KT, P], fp8, tag="wk")
                wv_s = consts.tile([128, KT, P], fp8, tag="wv")
                wo_s = consts.tile([128, 2, C], fp8, tag="wo")
                # DMA order tuned to the consumption schedule: wq+xt8 in
                # interleaved k-pairs (first DoubleRow matmul starts after
                # ~1/8), then wk, then the bf16 xt + wv for the (lagged) v
                # projection, rope tables, chunk-1 prefetches, rest.
                xt80_s = xt8s_pool.tile([128, KT, TCH], fp8, tag="xt8")
                for kq in range(0, KT, 2):
                    nc.sync.dma_start(
                        out=wq_s[:, kq:kq + 2, :],
                        in_=wq_d.ap()[kq:kq + 2].rearrange("k p n -> p k n"))
                    nc.sync.dma_start(
                        out=xt80_s[:, kq:kq + 2, :],
                        in_=xt8_d.ap()[kq:kq + 2, :, 0:TCH]
                        .rearrange("k p n -> p k n"))
                    nc.sync.dma_start(
                        out=wk_s[:, kq:kq + 2, :],
                        in_=wk_d.ap()[kq:kq + 2].rearrange("k p n -> p k n"))

                def xt8_tile(ch):
                    t = xt8s_pool.tile([128, KT, TCH], fp8, tag="xt8",
                                       name=f"xt8c{ch}")
                    nc.sync.dma_start(out=t,
                                      in_=xt8_d.ap()[:, :, ch * TCH:(ch + 1) * TCH]
                                      .rearrange("k p n -> p k n"))
                    return t

                def xt_tile(ch):
                    t = xts_pool.tile([128, KT, TCH], bf16, tag="xt",
                                      name=f"xtc{ch}")
                    nc.sync.dma_start(out=t,
                                      in_=xt_d.ap()[:, :, ch * TCH:(ch + 1) * TCH]
                                      .rearrange("k p n -> p k n"))
                    return t

                xt8s = [None] * NCH
                xts = [None] * NCH
                xt8s[0] = xt80_s
                xt8s[1] = xt8_tile(1)
                xt0_s = xts_pool.tile([128, KT, TCH], bf16, tag="xt")
                xts[0] = xt0_s
                for kq in range(0, KT, 4):
                    nc.sync.dma_start(
                        out=wv_s[:, kq:kq + 4, :],
                        in_=wv_d.ap()[kq:kq + 4].rearrange("k p n -> p k n"))
                    nc.sync.dma_start(
                        out=xt0_s[:, kq:kq + 4, :],
                        in_=xt_d.ap()[kq:kq + 4, :, 0:TCH]
                        .rearrange("k p n -> p k n"))
                cos_s = consts.tile([128, T], bf16, tag="cos")
                sin_s = consts.tile([128, T], bf16, tag="sin")
                nc.sync.dma_start(out=cos_s, in_=cos_d.ap())
                nc.sync.dma_start(out=sin_s, in_=sin_d.ap())
                if NCH > 1:
                    xts[1] = xt_tile(1)
                tri_s = consts.tile([128, 128], bf16, tag="tri")
                nc.sync.dma_start(out=tri_s, in_=tri_d.ap())
                nc.sync.dma_start(out=wo_s,
                                  in_=wo_d.ap().rearrange("k p n -> p k n"))
                ident = consts.tile([128, 128], bf16, tag="ident")
                make_identity(nc, ident[:])

                # ---- persistent activations
                qt_s = persist.tile([128, 2, NT], bf16, tag="qt")  # [d, head, tok]
                kt_s = persist.tile([128, 2, NT], bf16, tag="kt")
                v_s = persist.tile([128, NST, 2, 132], bf16, tag="v")
                nc.vector.memset(v_s[:, :, :, 128:129], 1.0)

                def v_proj(ch, xt_s):
                    t0 = ch * TCH
                    for st in range(TCH // 128):
                        stg = t0 // 128 + st
                        ps_v = psv.tile([128, P], f32, tag="psv")
                        for k in range(KT):
                            nc.tensor.matmul(
                                ps_v[:],
                                lhsT=xt_s[:, k, st * 128:(st + 1) * 128],
                                rhs=wv_s[:, k, :],
                                start=(k == 0), stop=(k == KT - 1))
                        # [t, (head d)] -> v_s[:, stg, head, 0:128]
                        nc.vector.tensor_copy(
                            v_s[:, stg, :, 0:128],
                            ps_v[:].rearrange("p (a d) -> p a d", a=2))

                for ch in range(NCH):
                    t0 = ch * TCH
                    pos0 = t0 % T          # position within batch
                    if ch >= 2:
                        xt8s[ch] = xt8_tile(ch)
                        xts[ch] = xt_tile(ch)
                    xt8_s = xt8s[ch]

                    # q/k projections (channel-major out, fp8 DoubleRow) + rope
                    for w_s, dst, qscale in ((wq_s, qt_s, float(beta)),
                                             (wk_s, kt_s, 1.0)):
                        for a in range(2):  # head within component
                            ps_q = ps1.tile([128, TCH], f32, tag="psq")
                            for k in range(0, KT, 2):
                                nc.tensor.matmul(
                                    ps_q[:],
                                    lhsT=w_s[:, k:k + 2, a * 128:(a + 1) * 128],
                                    rhs=xt8_s[:, k:k + 2, :],
                                    start=(k == 0), stop=(k == KT - 2),
                                    perf_mode=DR)
                            # rope: out = q * cos_dup + swap(q) * sin_signed
                            # (swap of partition halves must go through DMA --
                            # compute engines cannot move data across partitions)
                            # q's s_q*s_k/sqrt(HD) scale rides the Copy for free
                            q_sb = rope_pool.tile([128, TCH], bf16, tag="qsb")
                            nc.scalar.activation(
                                out=q_sb[:], in_=ps_q[:],
                                func=mybir.ActivationFunctionType.Copy,
                                scale=qscale)
                            qsw = rope_pool.tile([128, TCH], bf16, tag="qsw")
                            nc.gpsimd.dma_start(out=qsw[0:64, :], in_=q_sb[64:128, :])
                            nc.gpsimd.dma_start(out=qsw[64:128, :], in_=q_sb[0:64, :])
                            t1 = rope_pool.tile([128, TCH], bf16, tag="t1")
                            nc.vector.tensor_mul(
                                t1[:], q_sb[:], cos_s[:, pos0:pos0 + TCH])
                            t2 = rope_pool.tile([128, TCH], bf16, tag="t2")
                            nc.vector.tensor_mul(
                                t2[:], qsw[:], sin_s[:, pos0:pos0 + TCH])
                            nc.vector.tensor_add(
                                dst[:, a, t0:t0 + TCH], t1[:], t2[:])

                    # v projection lags one chunk (bf16 xt arrives later;
                    # the cold-start DMA window is bandwidth-bound)
                    if ch >= 1:
                        v_proj(ch - 1, xts[ch - 1])
                v_proj(NCH - 1, xts[NCH - 1])

            # ====== phases 2+3: causal attention + partial o-proj, per batch
            # o-proj for batch b is emitted right after batch b's attention,
            # so its matmuls fill attention-phase gaps and its 32MB output
            # DMA spreads over the rest of the kernel instead of the tail.
            ystages = {}
            MT = C // 128
            with (
                tc.tile_pool(name="pt", bufs=ST) as pt_pool,
                tc.tile_pool(name="att_small", bufs=4) as small_pool,
                tc.tile_pool(name="ysb", bufs=ST) as ysb_pool,
                tc.tile_pool(name="ostage", bufs=2) as o_pool,
                tc.tile_pool(name="ps_s", bufs=4, space="PSUM") as ps_s,
                tc.tile_pool(name="ps_y", bufs=2, space="PSUM") as ps_y,
                tc.tile_pool(name="ps_t", bufs=2, space="PSUM") as ps_t,
            ):
                def oproj_units(bb):
                    """Per-unit emission of batch bb's partial o-proj.

                    Each yielded step emits one [128, TCH] output tile's
                    matmul pair + PSUM->SBUF copy; one output DMA per TCH
                    chunk (16 tiles batched through a staging tile -- the
                    per-tile DMA trigger cost on the queue processor was
                    rate-limiting at ~0.64us/tile).
                    """
                    for lch in range(T // TCH):
                        lt0 = lch * TCH
                        t0 = bb * T + lt0
                        o_big = o_pool.tile([128, MT, TCH], bf16, tag="obig",
                                            name=f"obig{bb}{lch}")
                        for m in range(MT):
                            ps = ps_s.tile([128, 512], f32, tag="pss",
                                           name=f"psso{bb}{lch}{m}")
                            for k in range(2):           # cin k-tiles (= heads)
                                nc.tensor.matmul(
                                    ps[:, 0:TCH],
                                    lhsT=wo_s[:, k, m * 128:(m + 1) * 128],
                                    rhs=ystages[(bb, k)][:, lt0:lt0 + TCH],
                                    start=(k == 0), stop=(k == 1))
                            if m % 2 == 0:
                                nc.vector.tensor_copy(
                                    o_big[:, m, :], ps[:, 0:TCH])
                            else:
                                nc.scalar.copy(
                                    out=o_big[:, m, :], in_=ps[:, 0:TCH])
                            last_chunk = bb == B - 1 and lch == T // TCH - 1
                            if last_chunk and m >= MT - 4:
                                # the kernel-final tiles ship per-tile so the
                                # very last transfer is small
                                nc.sync.dma_start(
                                    out=out_d.ap()[m * 128:(m + 1) * 128,
                                                   t0:t0 + TCH],
                                    in_=o_big[:, m, :])
                            elif m % 4 == 3:
                                # sub-batched output DMA: 4 tiles per trigger
                                # overlaps the transfer with later copies, so
                                # the final chunk's 2MB doesn't drain serially
                                # at the kernel tail
                                nc.sync.dma_start(
                                    out=out_d.ap()[(m - 3) * 128:(m + 1) * 128,
                                                   t0:t0 + TCH]
                                    .rearrange("(mm p) n -> p mm n", mm=4),
                                    in_=o_big[:, m - 3:m + 1, :])
                            yield

                def attn_head(b, a, filler):
                    qh = qt_s[:, a, b * T:(b + 1) * T]   # [128, T]
                    kh = kt_s[:, a, b * T:(b + 1) * T]
                    y_stage = persist.tile([128, T], bf16,
                                           tag=f"ystage{b}{a}",
                                           name=f"ystage{b}{a}")
                    ystages[(b, a)] = y_stage
                    pts = [None] * ST
                    y_sbs = [None] * ST

                    def pv_tile(i):
                        # y tile i = sum_j P^T_j[:, tile i].T @ [v_j|1]
                        psy = ps_y.tile([128, 132], f32, tag="psy",
                                        name=f"psy{b}{a}{i}")
                        for j in range(i + 1):
                            nc.tensor.matmul(
                                psy[:, 0:129],
                                lhsT=pts[j][:, 128 * i:128 * (i + 1)],
                                rhs=v_s[:, b * ST + j, a, 0:129],
                                start=(j == 0), stop=(j == i))
                        recip = small_pool.tile([128, 1], f32, tag="recip",
                                                name=f"recip{b}{a}{i}")
                        nc.vector.reciprocal(recip[:], psy[:, 128:129])
                        y_sb = ysb_pool.tile([128, 128], bf16, tag="ysb",
                                             name=f"ysb{b}{a}{i}")
                        nc.vector.tensor_scalar_mul(
                            y_sb[:], psy[:, 0:128], recip[:])
                        y_sbs[i] = y_sb

                    # scores(j) interleaved with PV(j-1) and filler units:
                    # the exp stream on ACT is the throughput limit of the
                    # scores pipeline, so give the PE independent work
                    for j in range(ST):
                        pt_j = pt_pool.tile([128, T], bf16, tag="ptj",
                                            name=f"ptj{b}{a}{j}")
                        pts[j] = pt_j
                        q0 = 128 * j
                        while q0 < T:
                            w = min(512, T - q0)
                            ps = ps_s.tile([128, 512], f32, tag="pss",
                                           name=f"psss{b}{a}{j}{q0}")
                            nc.tensor.matmul(
                                ps[:, 0:w],
                                lhsT=kh[:, 128 * j:128 * (j + 1)],
                                rhs=qh[:, q0:q0 + w],
                                start=True, stop=True)
                            nc.scalar.activation(
                                out=pt_j[:, q0:q0 + w], in_=ps[:, 0:w],
                                func=mybir.ActivationFunctionType.Exp)
                            q0 += w
                        # causal mask on the diagonal 128x128 block
                        nc.vector.tensor_mul(
                            pt_j[:, 128 * j:128 * (j + 1)],
                            pt_j[:, 128 * j:128 * (j + 1)], tri_s[:])
                        if j >= 1:
                            pv_tile(j - 1)
                        if filler is not None:
                            for _ in range(2):
                                next(filler, None)
                    pv_tile(ST - 1)

                    # transposes after the PV chain, ascending = the order
                    # the DVE normalizes complete
                    for i in range(ST):
                        pst = ps_t.tile([128, 128], bf16, tag="pst",
                                        name=f"pst{b}{a}{i}")
                        nc.tensor.transpose(pst[:], y_sbs[i][:], ident[:])
                        # NB: must stay on DVE -- ACT reading bf16 PSUM
                        # hard-faulted the exec unit on HW
                        nc.vector.tensor_copy(
                            y_stage[:, 128 * i:128 * (i + 1)], pst[:])

                attn_head(0, 0, None)
                attn_head(0, 1, None)
                ofill = oproj_units(0)
                attn_head(1, 0, ofill)
                attn_head(1, 1, ofill)
                for _ in ofill:      # drain any remaining batch-0 units
                    pass
                for _ in oproj_units(1):
                    pass

    nc.compile()
    return nc


# ------------------------------------------------------------------ entrypoint
_NC_CACHE: dict = {}


def _get_nc(T: int, beta: float):
    key = (T, np.float32(beta).tobytes())
    if key not in _NC_CACHE:
        _NC_CACHE[key] = build_nc(T, beta=beta)
    return _NC_CACHE[key]


def assemble_output(results: list[dict], out_scale: float,
                    T: int = T_FULL) -> np.ndarray:
    # unshard = sum of the 8 tensor-parallel partial projections (bf16 -> f32)
    # scaled by the (host-folded) s_v * s_o ternary scales
    outT = results[0]["outt"].astype(np.float32)                # [C, NT]
    for r in results[1:]:
        outT += r["outt"].astype(np.float32)
    outT *= np.float32(out_scale)
    return np.ascontiguousarray(outT.T).reshape(B, T, C).astype(np.float32)


def kernel(**inputs) -> np.ndarray:
    in_maps, beta, out_scale = prep_inputs(inputs, T_FULL)
    nc = _get_nc(T_FULL, beta)
    res = run_bass_kernel_spmd(nc, in_maps, list(range(N_CORES)))
    return assemble_output(res.results, out_scale, T_FULL)


# revision 39
# speedup vs baseline: 1.0183x; 1.0183x over previous
"""Trainium2 Bass kernel for the octonion causal self-attention block.

Strategy (8 NeuronCores, SPMD):
  Each core owns one octonion component c (= heads 2c, 2c+1).
  - Host prep: ternary-quantize weights exactly as the reference does.
    q/k projections run in fp8 DoubleRow mode (2x PE throughput): the
    ternary weights are {-1,0,+1} -- exact in fp8 -- and x is cast to
    fp8 e4m3; the scalar scales (s_q*s_k/sqrt(HD)) fold into the rope
    cos/sin tables of q. v and o projections stay bf16 (fp8 there
    breaks the 2e-2 accuracy budget; verified by simulation).
  - Device phase 1: projections qT/kT (channel-major, fp8 DoubleRow) +
    v (natural, bf16), streaming xt once; RoPE applied on the fly. The
    v projection lags one chunk behind q/k so its (bigger) bf16 xt
    stream has time to arrive during the DMA-bound ramp.
  - Device phase 2: causal attention per (batch, head): S^T = K^T Q,
    exp (scores O(1) bounded), triangular mask on the diagonal tile,
    P^T V with an appended ones-column giving the softmax denominator,
    normalize, PE-transpose back to channel-major yT (kept in SBUF).
    scores(j) and PV(j-1) interleave so the PE keeps working while the
    ACT engine (the exp stream, ~1.7x slower than the scores matmuls)
    catches up.
  - Device phase 3: partial output projection with only local rows of
    Wo; the host sums the 8 partial [C, B*T] outputs while unsharding.
"""

import numpy as np
import ml_dtypes

import concourse.bass as bass
import concourse.tile as tile
from concourse import bacc, mybir
from concourse.bass_utils import run_bass_kernel_spmd
from concourse.masks import make_identity

# ---------------------------------------------------------------- problem dims
B, T_FULL, C, H = 2, 2048, 2048, 16
HD = C // H          # 128
P = C // 8           # 256
N_CORES = 8
KT = C // 128        # 16 contraction k-tiles

OCT_SIGN = np.array([
    [1, 1, 1, 1, 1, 1, 1, 1],
    [1,-1, 1,-1, 1,-1,-1, 1],
    [1,-1,-1, 1, 1, 1,-1,-1],
    [1, 1,-1,-1, 1,-1, 1,-1],
    [1,-1,-1,-1,-1, 1, 1, 1],
    [1, 1,-1, 1,-1,-1,-1, 1],
    [1, 1, 1,-1,-1, 1,-1,-1],
    [1,-1, 1, 1,-1,-1, 1,-1]], dtype=np.float32)
OCT_IDX = np.array([
    [0,1,2,3,4,5,6,7],
    [1,0,3,2,5,4,7,6],
    [2,3,0,1,6,7,4,5],
    [3,2,1,0,7,6,5,4],
    [4,5,6,7,0,1,2,3],
    [5,4,7,6,1,0,3,2],
    [6,7,4,5,2,3,0,1],
    [7,6,5,4,3,2,1,0]], dtype=np.int32)
_COMB = np.zeros((8, 8, 8), np.float32)
for _i in range(8):
    for _j in range(8):
        _COMB[OCT_IDX[_i, _j], _i, _j] = OCT_SIGN[_i, _j]

BF16 = ml_dtypes.bfloat16
FP8 = ml_dtypes.float8_e4m3   # TRN FP8_EXP4 (max +-240)


# ------------------------------------------------------------------- host prep
def _ternary_unit_scale(W: np.ndarray) -> tuple[np.ndarray, float]:
    """Reference's ternary quantization, split into unit {-1,0,1} + scale."""
    try:
        import jax
        import jax.numpy as jnp
        cpu = jax.local_devices(backend="cpu")[0]
        with jax.default_device(cpu):
            Wj = jnp.asarray(W)
            s = jnp.mean(jnp.abs(Wj)) + 1e-8
            u = jnp.round(jnp.clip(Wj / s, -1.0, 1.0))
            return np.asarray(u, np.float32), float(s)
    except Exception:
        s = np.float32(np.mean(np.abs(W.astype(np.float32)))) + np.float32(1e-8)
        return np.rint(np.clip(W / s, -1.0, 1.0)).astype(np.float32), float(s)


def _build_w_eff_unit(W: np.ndarray) -> tuple[np.ndarray, float]:
    """[8,P,P] weights -> effective unit-ternary [C, C] + scalar scale."""
    u, s = _ternary_unit_scale(W)
    eff = np.einsum("kij,ipq->jpkq", _COMB, u).reshape(C, C).astype(np.float32)
    return eff, s


def _rope_colperm() -> np.ndarray:
    """colperm[new] = old: within each head, [re0..re63 | im0..im63]."""
    perm = np.zeros(C, dtype=np.int64)
    for h in range(H):
        base = h * HD
        for r in range(HD // 2):
            perm[base + r] = base + 2 * r
            perm[base + HD // 2 + r] = base + 2 * r + 1
    return perm


def _dup_half_tables(cos: np.ndarray, sin: np.ndarray, scale: float):
    """RoPE tables in duplicated-half layout [128, T], bf16."""
    Tn = cos.shape[0]
    cosd = np.empty((128, Tn), np.float32)
    cosd[0:64] = cos.T
    cosd[64:128] = cos.T
    sind = np.empty((128, Tn), np.float32)
    sind[0:64] = -sin.T
    sind[64:128] = sin.T
    return (cosd * scale).astype(BF16), (sind * scale).astype(BF16)


def prep_inputs(inputs: dict, T: int) -> list[dict]:
    """Build the 8 per-core input maps from the full problem inputs."""
    NT = B * T
    x = np.asarray(inputs["x"], np.float32)[:, :T, :]
    cos = np.asarray(inputs["freqs_cos"], np.float32)[:T]   # [T, 64]
    sin = np.asarray(inputs["freqs_sin"], np.float32)[:T]

    wq_eff, s_q = _build_w_eff_unit(np.asarray(inputs["wq"], np.float32))
    wk_eff, s_k = _build_w_eff_unit(np.asarray(inputs["wk"], np.float32))
    wv_eff, s_v = _build_w_eff_unit(np.asarray(inputs["wv"], np.float32))
    wo_eff, s_o = _build_w_eff_unit(np.asarray(inputs["wo"], np.float32))
    # all four weight sets stay unit-ternary {-1,0,+1} (exact in fp8).
    # s_q*s_k/sqrt(HD) folds into the q PSUM->SBUF copy scale (compile-time
    # constant); s_v*s_o folds into the host-side unshard sum.

    perm = _rope_colperm()
    wq_eff = wq_eff[:, perm]
    wk_eff = wk_eff[:, perm]
    beta = float(s_q) * float(s_k) / float(np.sqrt(HD))
    out_scale = float(s_v) * float(s_o)

    xf = x.reshape(NT, C).T          # [C, NT]
    xt = np.ascontiguousarray(xf.reshape(KT, 128, NT).astype(BF16))
    xt8 = np.ascontiguousarray(xf.reshape(KT, 128, NT).astype(FP8))

    cosd, sind = _dup_half_tables(cos, sin, 1.0)

    tri = np.triu(np.ones((128, 128), np.float32)).astype(BF16)  # [s,q] s<=q

    def blocks8(w_eff: np.ndarray, c: int) -> np.ndarray:
        blk = w_eff[:, c * P:(c + 1) * P]                  # [C, 256]
        return np.ascontiguousarray(blk.reshape(KT, 128, P).astype(FP8))

    in_maps = []
    for c in range(N_CORES):
        wo_rows = np.ascontiguousarray(
            wo_eff[c * P:(c + 1) * P, :].reshape(2, 128, C).astype(FP8))
        in_maps.append({
            "xt": xt,
            "xt8": xt8,
            "wq": blocks8(wq_eff, c),
            "wk": blocks8(wk_eff, c),
            "wv": blocks8(wv_eff, c),
            "wo": wo_rows,
            "cosd": cosd, "sind": sind,
            "tri": tri,
        })
    return in_maps, beta, out_scale


# ------------------------------------------------------------- device program
def build_nc(T: int = T_FULL, n_cores: int = N_CORES, beta: float = 1.0):
    NT = B * T
    ST = T // 128            # s-tiles per batch
    NST = NT // 128
    TCH = min(512, T)        # token chunk; must not cross a batch boundary
    NCH = NT // TCH
    bf16 = mybir.dt.bfloat16
    fp8 = mybir.dt.float8e4
    f32 = mybir.dt.float32
    DR = mybir.MatmulPerfMode.DoubleRow

    nc = bacc.Bacc("TRN2", target_bir_lowering=False, debug=False,
                   num_devices=n_cores)

    xt_d = nc.dram_tensor("xt", [KT, 128, NT], bf16, kind="ExternalInput")
    xt8_d = nc.dram_tensor("xt8", [KT, 128, NT], fp8, kind="ExternalInput")
    wq_d = nc.dram_tensor("wq", [KT, 128, P], fp8, kind="ExternalInput")
    wk_d = nc.dram_tensor("wk", [KT, 128, P], fp8, kind="ExternalInput")
    wv_d = nc.dram_tensor("wv", [KT, 128, P], fp8, kind="ExternalInput")
    wo_d = nc.dram_tensor("wo", [2, 128, C], fp8, kind="ExternalInput")
    cos_d = nc.dram_tensor("cosd", [128, T], bf16, kind="ExternalInput")
    sin_d = nc.dram_tensor("sind", [128, T], bf16, kind="ExternalInput")
    tri_d = nc.dram_tensor("tri", [128, 128], bf16, kind="ExternalInput")
    out_d = nc.dram_tensor("outt", [C, NT], bf16, kind="ExternalOutput")

    with tile.TileContext(nc) as tc:
        with (
            tc.tile_pool(name="consts", bufs=1) as consts,
            tc.tile_pool(name="persist", bufs=1) as persist,
        ):
            # ================= phase 1: projections + rope =================
            with (
                nc.named_scope("proj"),
                tc.tile_pool(name="xts", bufs=3) as xts_pool,
                tc.tile_pool(name="xt8s", bufs=2) as xt8s_pool,
                tc.tile_pool(name="rope", bufs=6) as rope_pool,
                tc.tile_pool(name="ps1", bufs=3, space="PSUM") as ps1,
                tc.tile_pool(name="psv", bufs=3, space="PSUM") as psv,
            ):
                # ---- resident constants
                wq_s = consts.tile([128, KT, P], fp8, tag="wq")
                wk_s = consts.tile([128, KT, P], fp8, tag="wk")
                wv_s = consts.tile([128, KT, P], fp8, tag="wv")
                wo_s = consts.tile([128, 2, C], fp8, tag="wo")
                # DMA order tuned to the consumption schedule: wq+xt8 in
                # interleaved k-pairs (first DoubleRow matmul starts after
                # ~1/8), then wk, then the bf16 xt + wv for the (lagged) v
                # projection, rope tables, chunk-1 prefetches, rest.
                xt80_s = xt8s_pool.tile([128, KT, TCH], fp8, tag="xt8")
                for kq in range(0, KT, 2):
                    nc.sync.dma_start(
                        out=wq_s[:, kq:kq + 2, :],
                        in_=wq_d.ap()[kq:kq + 2].rearrange("k p n -> p k n"))
                    nc.sync.dma_start(
                        out=xt80_s[:, kq:kq + 2, :],
                        in_=xt8_d.ap()[kq:kq + 2, :, 0:TCH]
                        .rearrange("k p n -> p k n"))
                nc.sync.dma_start(out=wk_s,
                                  in_=wk_d.ap().rearrange("k p n -> p k n"))

                def xt8_tile(ch):
                    t = xt8s_pool.tile([128, KT, TCH], fp8, tag="xt8",
                                       name=f"xt8c{ch}")
                    nc.sync.dma_start(out=t,
                                      in_=xt8_d.ap()[:, :, ch * TCH:(ch + 1) * TCH]
                                      .rearrange("k p n -> p k n"))
                    return t

                def xt_tile(ch):
                    t = xts_pool.tile([128, KT, TCH], bf16, tag="xt",
                                      name=f"xtc{ch}")
                    nc.sync.dma_start(out=t,
                                      in_=xt_d.ap()[:, :, ch * TCH:(ch + 1) * TCH]
                                      .rearrange("k p n -> p k n"))
                    return t

                xt8s = [None] * NCH
                xts = [None] * NCH
                xt8s[0] = xt80_s
                xt8s[1] = xt8_tile(1)
                xt0_s = xts_pool.tile([128, KT, TCH], bf16, tag="xt")
                xts[0] = xt0_s
                for kq in range(0, KT, 4):
                    nc.sync.dma_start(
                        out=wv_s[:, kq:kq + 4, :],
                        in_=wv_d.ap()[kq:kq + 4].rearrange("k p n -> p k n"))
                    nc.sync.dma_start(
                        out=xt0_s[:, kq:kq + 4, :],
                        in_=xt_d.ap()[kq:kq + 4, :, 0:TCH]
                        .rearrange("k p n -> p k n"))
                cos_s = consts.tile([128, T], bf16, tag="cos")
                sin_s = consts.tile([128, T], bf16, tag="sin")
                nc.sync.dma_start(out=cos_s, in_=cos_d.ap())
                nc.sync.dma_start(out=sin_s, in_=sin_d.ap())
                if NCH > 1:
                    xts[1] = xt_tile(1)
                tri_s = consts.tile([128, 128], bf16, tag="tri")
                nc.sync.dma_start(out=tri_s, in_=tri_d.ap())
                nc.sync.dma_start(out=wo_s,
                                  in_=wo_d.ap().rearrange("k p n -> p k n"))
                ident = consts.tile([128, 128], bf16, tag="ident")
                make_identity(nc, ident[:])

                # ---- persistent activations
                qt_s = persist.tile([128, 2, NT], bf16, tag="qt")  # [d, head, tok]
                kt_s = persist.tile([128, 2, NT], bf16, tag="kt")
                v_s = persist.tile([128, NST, 2, 132], bf16, tag="v")
                nc.vector.memset(v_s[:, :, :, 128:129], 1.0)

                def v_proj(ch, xt_s):
                    t0 = ch * TCH
                    for st in range(TCH // 128):
                        stg = t0 // 128 + st
                        ps_v = psv.tile([128, P], f32, tag="psv")
                        for k in range(KT):
                            nc.tensor.matmul(
                                ps_v[:],
                                lhsT=xt_s[:, k, st * 128:(st + 1) * 128],
                                rhs=wv_s[:, k, :],
                                start=(k == 0), stop=(k == KT - 1))
                        # [t, (head d)] -> v_s[:, stg, head, 0:128]
                        nc.vector.tensor_copy(
                            v_s[:, stg, :, 0:128],
                            ps_v[:].rearrange("p (a d) -> p a d", a=2))

                for ch in range(NCH):
                    t0 = ch * TCH
                    pos0 = t0 % T          # position within batch
                    if ch >= 2:
                        xt8s[ch] = xt8_tile(ch)
                        xts[ch] = xt_tile(ch)
                    xt8_s = xt8s[ch]

                    # q/k projections (channel-major out, fp8 DoubleRow) + rope
                    for w_s, dst, qscale in ((wq_s, qt_s, float(beta)),
                                             (wk_s, kt_s, 1.0)):
                        for a in range(2):  # head within component
                            ps_q = ps1.tile([128, TCH], f32, tag="psq")
                            for k in range(0, KT, 2):
                                nc.tensor.matmul(
                                    ps_q[:],
                                    lhsT=w_s[:, k:k + 2, a * 128:(a + 1) * 128],
                                    rhs=xt8_s[:, k:k + 2, :],
                                    start=(k == 0), stop=(k == KT - 2),
                                    perf_mode=DR)
                            # rope: out = q * cos_dup + swap(q) * sin_signed
                            # (swap of partition halves must go through DMA --
                            # compute engines cannot move data across partitions)
                            # q's s_q*s_k/sqrt(HD) scale rides the Copy for free
                            q_sb = rope_pool.tile([128, TCH], bf16, tag="qsb")
                            nc.scalar.activation(
                                out=q_sb[:], in_=ps_q[:],
                                func=mybir.ActivationFunctionType.Copy,
                                scale=qscale)
                            qsw = rope_pool.tile([128, TCH], bf16, tag="qsw")
                            nc.gpsimd.dma_start(out=qsw[0:64, :], in_=q_sb[64:128, :])
                            nc.gpsimd.dma_start(out=qsw[64:128, :], in_=q_sb[0:64, :])
                            t1 = rope_pool.tile([128, TCH], bf16, tag="t1")
                            nc.vector.tensor_mul(
                                t1[:], q_sb[:], cos_s[:, pos0:pos0 + TCH])
                            t2 = rope_pool.tile([128, TCH], bf16, tag="t2")
                            nc.vector.tensor_mul(
                                t2[:], qsw[:], sin_s[:, pos0:pos0 + TCH])
                            nc.vector.tensor_add(
                                dst[:, a, t0:t0 + TCH], t1[:], t2[:])

                    # v projection lags one chunk (bf16 xt arrives later;
                    # the cold-start DMA window is bandwidth-bound)
                    if ch >= 1:
                        v_proj(ch - 1, xts[ch - 1])
                v_proj(NCH - 1, xts[NCH - 1])

            # ====== phases 2+3: causal attention + partial o-proj, per batch
            # o-proj for batch b is emitted right after batch b's attention,
            # so its matmuls fill attention-phase gaps and its 32MB output
            # DMA spreads over the rest of the kernel instead of the tail.
            ystages = {}
            MT = C // 128
            with (
                tc.tile_pool(name="pt", bufs=ST) as pt_pool,
                tc.tile_pool(name="att_small", bufs=4) as small_pool,
                tc.tile_pool(name="ysb", bufs=ST) as ysb_pool,
                tc.tile_pool(name="ostage", bufs=2) as o_pool,
                tc.tile_pool(name="ps_s", bufs=4, space="PSUM") as ps_s,
                tc.tile_pool(name="ps_y", bufs=2, space="PSUM") as ps_y,
                tc.tile_pool(name="ps_t", bufs=2, space="PSUM") as ps_t,
            ):
                def oproj_units(bb):
                    """Per-unit emission of batch bb's partial o-proj.

                    Each yielded step emits one [128, TCH] output tile's
                    matmul pair + PSUM->SBUF copy; one output DMA per TCH
                    chunk (16 tiles batched through a staging tile -- the
                    per-tile DMA trigger cost on the queue processor was
                    rate-limiting at ~0.64us/tile).
                    """
                    for lch in range(T // TCH):
                        lt0 = lch * TCH
                        t0 = bb * T + lt0
                        o_big = o_pool.tile([128, MT, TCH], bf16, tag="obig",
                                            name=f"obig{bb}{lch}")
                        for m in range(MT):
                            ps = ps_s.tile([128, 512], f32, tag="pss",
                                           name=f"psso{bb}{lch}{m}")
                            for k in range(2):           # cin k-tiles (= heads)
                                nc.tensor.matmul(
                                    ps[:, 0:TCH],
                                    lhsT=wo_s[:, k, m * 128:(m + 1) * 128],
                                    rhs=ystages[(bb, k)][:, lt0:lt0 + TCH],
                                    start=(k == 0), stop=(k == 1))
                            if m % 2 == 0:
                                nc.vector.tensor_copy(
                                    o_big[:, m, :], ps[:, 0:TCH])
                            else:
                                nc.scalar.copy(
                                    out=o_big[:, m, :], in_=ps[:, 0:TCH])
                            if m % 4 == 3:
                                # sub-batched output DMA: 4 tiles per trigger
                                # overlaps the transfer with later copies, so
                                # the final chunk's 2MB doesn't drain serially
                                # at the kernel tail
                                nc.sync.dma_start(
                                    out=out_d.ap()[(m - 3) * 128:(m + 1) * 128,
                                                   t0:t0 + TCH]
                                    .rearrange("(mm p) n -> p mm n", mm=4),
                                    in_=o_big[:, m - 3:m + 1, :])
                            yield

                def attn_head(b, a, filler):
                    qh = qt_s[:, a, b * T:(b + 1) * T]   # [128, T]
                    kh = kt_s[:, a, b * T:(b + 1) * T]
                    y_stage = persist.tile([128, T], bf16,
                                           tag=f"ystage{b}{a}",
                                           name=f"ystage{b}{a}")
                    ystages[(b, a)] = y_stage
                    pts = [None] * ST
                    y_sbs = [None] * ST

                    def pv_tile(i):
                        # y tile i = sum_j P^T_j[:, tile i].T @ [v_j|1]
                        psy = ps_y.tile([128, 132], f32, tag="psy",
                                        name=f"psy{b}{a}{i}")
                        for j in range(i + 1):
                            nc.tensor.matmul(
                                psy[:, 0:129],
                                lhsT=pts[j][:, 128 * i:128 * (i + 1)],
                                rhs=v_s[:, b * ST + j, a, 0:129],
                                start=(j == 0), stop=(j == i))
                        recip = small_pool.tile([128, 1], f32, tag="recip",
                                                name=f"recip{b}{a}{i}")
                        nc.vector.reciprocal(recip[:], psy[:, 128:129])
                        y_sb = ysb_pool.tile([128, 128], bf16, tag="ysb",
                                             name=f"ysb{b}{a}{i}")
                        nc.vector.tensor_scalar_mul(
                            y_sb[:], psy[:, 0:128], recip[:])
                        y_sbs[i] = y_sb

                    # scores(j) interleaved with PV(j-1) and filler units:
                    # the exp stream on ACT is the throughput limit of the
                    # scores pipeline, so give the PE independent work
                    for j in range(ST):
                        pt_j = pt_pool.tile([128, T], bf16, tag="ptj",
                                            name=f"ptj{b}{a}{j}")
                        pts[j] = pt_j
                        q0 = 128 * j
                        while q0 < T:
                            w = min(512, T - q0)
                            ps = ps_s.tile([128, 512], f32, tag="pss",
                                           name=f"psss{b}{a}{j}{q0}")
                            nc.tensor.matmul(
                                ps[:, 0:w],
                                lhsT=kh[:, 128 * j:128 * (j + 1)],
                                rhs=qh[:, q0:q0 + w],
                                start=True, stop=True)
                            nc.scalar.activation(
                                out=pt_j[:, q0:q0 + w], in_=ps[:, 0:w],
                                func=mybir.ActivationFunctionType.Exp)
                            q0 += w
                        # causal mask on the diagonal 128x128 block
                        nc.vector.tensor_mul(
                            pt_j[:, 128 * j:128 * (j + 1)],
                            pt_j[:, 128 * j:128 * (j + 1)], tri_s[:])
                        if j >= 1:
                            pv_tile(j - 1)
                        if filler is not None:
                            for _ in range(2):
                                next(filler, None)
                    pv_tile(ST - 1)

                    # transposes after the PV chain, ascending = the order
                    # the DVE normalizes complete
                    for i in range(ST):
                        pst = ps_t.tile([128, 128], bf16, tag="pst",
                                        name=f"pst{b}{a}{i}")
                        nc.tensor.transpose(pst[:], y_sbs[i][:], ident[:])
                        # NB: must stay on DVE -- ACT reading bf16 PSUM
                        # hard-faulted the exec unit on HW
                        nc.vector.tensor_copy(
                            y_stage[:, 128 * i:128 * (i + 1)], pst[:])

                attn_head(0, 0, None)
                attn_head(0, 1, None)
                ofill = oproj_units(0)
                attn_head(1, 0, ofill)
                attn_head(1, 1, ofill)
                for _ in ofill:      # drain any remaining batch-0 units
                    pass
                for _ in oproj_units(1):
                    pass

    nc.compile()
    return nc


# ------------------------------------------------------------------ entrypoint
_NC_CACHE: dict = {}


def _get_nc(T: int, beta: float):
    key = (T, np.float32(beta).tobytes())
    if key not in _NC_CACHE:
        _NC_CACHE[key] = build_nc(T, beta=beta)
    return _NC_CACHE[key]


def assemble_output(results: list[dict], out_scale: float,
                    T: int = T_FULL) -> np.ndarray:
    # unshard = sum of the 8 tensor-parallel partial projections (bf16 -> f32)
    # scaled by the (host-folded) s_v * s_o ternary scales
    outT = results[0]["outt"].astype(np.float32)                # [C, NT]
    for r in results[1:]:
        outT += r["outt"].astype(np.float32)
    outT *= np.float32(out_scale)
    return np.ascontiguousarray(outT.T).reshape(B, T, C).astype(np.float32)


def kernel(**inputs) -> np.ndarray:
    in_maps, beta, out_scale = prep_inputs(inputs, T_FULL)
    nc = _get_nc(T_FULL, beta)
    res = run_bass_kernel_spmd(nc, in_maps, list(range(N_CORES)))
    return assemble_output(res.results, out_scale, T_FULL)
